# revision 8
# baseline (speedup 1.0000x reference)
"""Trainium2 Bass kernel for nn_BigAttention (weight-norm MLP + softmax-over-k).

Math (per the reference):
    W1e = g1 * W1 / ||W1||_F          [1024, 3072]
    W2e = g2 * W2 / ||W2||_F          [1, 1024]
    hv  = v @ W1e[:, :2048].T         [B,K,N,1024]
    hq  = q @ W1e[:, 2048:].T         [B,K,1024]
    joint  = relu(hv + hq + b1)
    logits = joint @ W2e.T  (+ b2, which cancels in the softmax over k)
    out = softmax(logits, axis=K)     [B,K,N,1]

Sharding: data-parallel over batch, 8 batches per core; weights replicated.

The dominant v-matmuls and the hq matmuls run fp8 e4m3 with
perf_mode=DoubleRow: two 128-deep contraction chunks fuse into one PE
instruction streaming 2 cols/cycle (~2x bf16). W1 is scaled by S1 host-side
so its tiny entries survive e4m3; hq/b1 carry the same scale and W2 carries
1/S1 (relu commutes with positive scaling). The one-hot hq-add closers
contract only 96 bk-partitions, where DoubleRow cannot cut streamed columns
-- they stay bf16 (FWL-eligible stationary).

Per-core device program (rows r = (b_local, k, n) flattened, R = 3456):
  - warmup: a short burst of dummy matmuls on zeroed SBUF rides the initial
    DMA wait so the PE's HAM clock-gate is at full rate when real data lands.
  - hq[96, 1024] via 8 DoubleRow matmuls; b1 added on the PSUM->SBUF copy.
  - main: per 128-row tile, PSUM[row, hidden 1024] accumulates 8 paired
    v^T-chunk DoubleRow matmuls plus two bf16 one-hot closers adding
    hq[bk(row), :].
  - epilogue per tile: one DVE scalar_tensor_tensor computes
    (PSUM max 0) * w2_broadcast with accum_out = per-row sum = the logit.
  - softmax over k, three stages so only bk 92..95 remain after the last
    tile: logits go [128, t] -> StreamTranspose -> linear DRAM ->
    [96 (b,k), 36 n] SBUF; exp on ACT; per-(b,n) sums accumulate on the PE
    across stages; reciprocal broadcast back via one-hot matmul; final
    scale on DVE; one strided DMA writes the [8,12,36,1] output slice.

All heavy inputs are host-repacked "partition-major" so every big DMA is 128
contiguous runs (one per partition). DMA issue order matches consumption
order, with the first v/W1v pieces split small so the first real matmul can
start as early as possible.
"""

import ml_dtypes
import numpy as np

import concourse.bacc as bacc
import concourse.mybir as mybir
import concourse.tile as tile
from concourse.bass_utils import run_bass_kernel_spmd

F32 = mybir.dt.float32
NCORES = 8
B, K, N = 64, 12, 36
VD, QD, HID = 2048, 1024, 1024
BL = B // NCORES              # local batches per core
R = BL * K * N                # 3456 rows per core
BK = BL * K                   # 96 (b,k) groups per core
CC = VD // 128                # 16 contraction chunks over v-dim
QC = QD // 128                # 8 contraction chunks over q-dim
RC = 384                      # rows per DMA chunk (9 chunks)
NCH = R // RC
RT = 128                      # rows per PSUM tile
NT = RC // RT
NRT = R // RT                 # 27 row tiles

_NC_CACHE = None

MM_DT = mybir.dt.float32r
BF16 = mybir.dt.bfloat16
FP8 = mybir.dt.float8e4
S1 = 2048.0

# cc-chunk DMA granularity per v chunk: chunk 0 lands in four small pieces
# so the first matmul pair can start early; later chunks use two fat DMAs.
VSPLITS0 = (2, 2, 2, 2, 4, 4)
VSPLITS = (8, 8)
WSPLITS = (2, 2, 4, 4, 4)     # w1v chunk groups (first ones small)


def _splits_index(splits):
    idx = {}
    base = 0
    for j, g in enumerate(splits):
        for o in range(g):
            idx[base + o] = (j, o)
        base += g
    return idx


def _build_nc():
    nc = bacc.Bacc("TRN2", target_bir_lowering=False, debug=False,
                   num_devices=NCORES)

    DR = mybir.MatmulPerfMode.DoubleRow

    def mm(out, lhsT, rhs, **kw):
        nc.tensor.matmul(out, lhsT, rhs, **kw)

    w1vt = nc.dram_tensor("w1vt", [128, CC, HID], FP8, kind="ExternalInput").ap()
    # qt and W1q^T packed along the free dim: [:, cq, 0:96]=q^T, [:, cq, 96:1120]=W1q^T
    qtwq = nc.dram_tensor("qtwq", [128, QC, BK + HID], FP8, kind="ExternalInput").ap()
    # fp32r one-hot row-selection matrix for the hq-add closers; bf16 was
    # measured to slow the whole PE stream ~20% (FWL/DoubleRow weight-path
    # mode thrash), fp32r keeps the mains at full rate
    oneh_d = nc.dram_tensor("oneh", [BK, R], MM_DT, kind="ExternalInput").ap()
    # small constants: W2e/S1 replicated (bf16), b1*S1 replicated (bf16),
    # softmax selection matrices (fp32)
    w2b = nc.dram_tensor("w2b", [128, HID], BF16, kind="ExternalInput").ap()
    b1b = nc.dram_tensor("b1b", [BK, HID], BF16, kind="ExternalInput").ap()
    sel = nc.dram_tensor("sel", [BK, BL + BK + BL], F32, kind="ExternalInput").ap()
    # v is split: the first two chunks ride with the weights at the front of
    # the upload order; the bulk uploads last, hidden under early compute.
    vth = nc.dram_tensor("vth", [2, 128, CC, RC], FP8, kind="ExternalInput").ap()
    vtr = nc.dram_tensor("vtr", [NCH - 2, 128, CC, RC], FP8, kind="ExternalInput").ap()
    out = nc.dram_tensor("out", [BL, K, N, 1], F32, kind="ExternalOutput").ap()

    MAX = mybir.AluOpType.max
    MULT = mybir.AluOpType.mult
    BYPASS = mybir.AluOpType.bypass
    ADD = mybir.AluOpType.add

    vidx0 = _splits_index(VSPLITS0)
    vidx = _splits_index(VSPLITS)
    widx = _splits_index(WSPLITS)

    with tile.TileContext(nc) as tc:
        with tc.tile_pool(name="const", bufs=1) as cpool, \
             tc.tile_pool(name="wv", bufs=1) as wvpool, \
             tc.tile_pool(name="vtp", bufs=2) as vtpool, \
             tc.tile_pool(name="work", bufs=3) as work, \
             tc.tile_pool(name="small", bufs=1) as small, \
             tc.tile_pool(name="dram", bufs=1, space="DRAM") as dpool, \
             tc.tile_pool(name="psum", bufs=4, space="PSUM") as pspool:

            # ---- PE warmup burst: ~10 dummy matmuls on zeroed SBUF keep the
            # HAM clock-gate ramping while the first real operands upload.
            wdum = small.tile([128, 2, 128], FP8)
            nc.vector.memset(wdum, 0.0)
            xdum = small.tile([128, 2, 512], FP8)
            nc.vector.memset(xdum, 0.0)
            psd = pspool.tile([128, 512], F32, tag="sm", bufs=2)
            for _ in range(14):
                mm(psd, wdum, xdum, start=True, stop=True, perf_mode=DR)

            # ---- startup DMAs; issue order matches consumption order: the
            # first matmul pair needs v chunk 0 piece 0 + w1v group 0.
            def vt_chunk_tiles(ch):
                src_ap = vth[ch] if ch < 2 else vtr[ch - 2]
                splits = VSPLITS0 if ch == 0 else VSPLITS
                tiles = []
                base = 0
                for j, g in enumerate(splits):
                    t = vtpool.tile([128, g, RC], FP8, tag=f"vt{j}_{g}")
                    nc.sync.dma_start(out=t, in_=src_ap[:, base:base + g, :])
                    tiles.append(t)
                    base += g
                return tiles

            wv_g = []

            def wv_group(j):
                base = sum(WSPLITS[:j])
                g = WSPLITS[j]
                t = wvpool.tile([128, g, HID], FP8, tag=f"wvg{j}")
                nc.scalar.dma_start(out=t, in_=w1vt[:, base:base + g, :])
                wv_g.append(t)

            vt_cur = vt_chunk_tiles(0)
            for j in range(len(WSPLITS)):
                wv_group(j)

            qtwq_s = cpool.tile([128, QC, BK + HID], FP8)
            nc.sync.dma_start(out=qtwq_s, in_=qtwq)

            vt_next = vt_chunk_tiles(1)

            w2b_s = cpool.tile([128, HID], BF16)
            nc.scalar.dma_start(out=w2b_s, in_=w2b)
            b1b_s = cpool.tile([BK, HID], BF16)
            nc.gpsimd.dma_start(out=b1b_s, in_=b1b)
            sel_s = cpool.tile([BK, BL + BK + BL], F32)
            nc.gpsimd.dma_start(out=sel_s, in_=sel)

            oneh_s = cpool.tile([BK, R], MM_DT)
            nc.gpsimd.dma_start(out=oneh_s, in_=oneh_d)

            selb_s = sel_s[:, 0:BL]
            selbt_s = sel_s[0:BL, BL:BL + BK]
            # selb with bk rows < 92 zeroed: lets the tail sums matmul
            # contract from base partition 64 (hw allows 0/32/64 only)
            selm_s = sel_s[:, BL + BK:BL + BK + BL]

            # per-row logits, laid out [p, rt] with row = rt*128 + p, split
            # into two tiles so earlier flushes hide under the main loop.
            # 32 columns (StreamTranspose needs 32x32 blocks).
            NRT_A = 18   # 18*128 rows = 64 (b,k) groups — a 32-aligned bk split
            ls_a = cpool.tile([128, 32], F32)
            nc.vector.memset(ls_a, 0.0)
            ls_b = cpool.tile([128, 32], F32)
            nc.vector.memset(ls_b, 0.0)
            lg = dpool.tile([R], F32)
            lg2 = lg.rearrange("(t p) -> t p", t=NRT, p=128)

            def flush_logits(ls, ls_t_name, t0, t1):
                # ls[p, t - t0] holds L[t*128 + p] for t in [t0, t1)
                ls_t = cpool.tile([128, 32], F32, name=ls_t_name)
                nc.vector.transpose(ls_t, ls)
                # four rings generate descriptors in parallel (the ~0.7us
                # DIRECT2D gen per DMA is the dominant serial tail cost)
                engs = (nc.sync, nc.scalar, nc.gpsimd, nc.scalar)
                for i in range(4):
                    engs[i].dma_start(
                        out=lg2[t0:t1, 32 * i:32 * i + 32],
                        in_=ls_t[32 * i:32 * i + (t1 - t0), :])

            hq_s = cpool.tile([BK, HID], MM_DT)
            s96 = small.tile([BK, N], F32)
            e96 = small.tile([BK, N], F32)
            sums_ps = pspool.tile([BL, N], F32, tag="sm", bufs=2)

            def softmax_head(bk0, bk1, start, stop, selap=None, mm0=None):
                # exp + partial per-(b,n) sums for bk rows [bk0, bk1)
                m0 = bk0 if mm0 is None else mm0
                nc.sync.dma_start(
                    out=s96[bk0:bk1, :],
                    in_=lg.rearrange("(bk n) -> bk n", n=N)[bk0:bk1, :])
                nc.scalar.activation(e96[bk0:bk1, :], s96[bk0:bk1, :],
                                     mybir.ActivationFunctionType.Exp)
                sl = selb_s if selap is None else selap
                mm(sums_ps, sl[m0:bk1, :], e96[m0:bk1, :],
                   start=start, stop=stop)

            def emit_vmms(t, ps):
                # fp8 DoubleRow: each matmul contracts TWO 128-deep v chunks
                # (lhsT [128, 2, 128 rows], rhs [128, 2, 512]) at 2 cols/cycle.
                for cc in range(0, CC, 2):
                    vj, vo = (vidx0 if vt_cur is vt_c0 else vidx)[cc]
                    lhsT = vt_cur[vj][:, vo:vo + 2, t * RT:(t + 1) * RT]
                    wj, wo = widx[cc]
                    wvc = wv_g[wj][:, wo:wo + 2, :]
                    mm(ps[:, 0:512], lhsT, wvc[:, :, 0:512],
                       start=(cc == 0), stop=False, perf_mode=DR)
                    mm(ps[:, 512:1024], lhsT, wvc[:, :, 512:1024],
                       start=(cc == 0), stop=False, perf_mode=DR)

            def emit_closer(rt, ps):
                oh = oneh_s[:, rt * RT:(rt + 1) * RT]
                mm(ps[:, 0:512], oh, hq_s[:, 0:512], start=False, stop=True)
                mm(ps[:, 512:1024], oh, hq_s[:, 512:1024],
                   start=False, stop=True)
                relu_w2 = work.tile([128, HID], F32, tag="relu_w2")
                ls, col = (ls_a, rt) if rt < NRT_A else (ls_b, rt - NRT_A)
                nc.vector.scalar_tensor_tensor(
                    out=relu_w2, in0=ps, scalar=0.0, in1=w2b_s,
                    op0=MAX, op1=MULT,
                    accum_out=ls[:, col:col + 1])
                if rt == NRT_A - 1:
                    # flush + softmax head for bk 0:64 under the main loop
                    flush_logits(ls_a, "ls_ta", 0, NRT_A)
                    softmax_head(0, 64, True, False)
                elif rt == NRT - 2:
                    # bk 64:92 are complete through tile 25; only bk 92:96
                    # remain for the post-loop tail
                    flush_logits(ls_b, "ls_tb", NRT_A, NRT - 1)
                    softmax_head(64, 92, False, False)

            # ---- chunk 0: v-matmuls for tiles 0..2 first, then hq (its DMAs
            # arrive under the v work), then the deferred closers.
            vt_c0 = vt_cur
            ps0 = []
            for t in range(NT):
                ps = pspool.tile([128, HID], F32, tag="ps", bufs=3)
                emit_vmms(t, ps)
                ps0.append(ps)

            # hq[96, 1024] via fp8 DoubleRow over q-chunk pairs (out partition
            # 96 = lhsT free 192 / 2); b1 (scaled) added on the PSUM -> SBUF
            # copy, stored bf16 for the closers.
            for hh in range(2):
                hs = slice(hh * 512, (hh + 1) * 512)
                ps_q = pspool.tile([BK, 512], F32, tag="sm", bufs=2,
                                   name=f"hq_ps{hh}")
                for cq in range(0, QC, 2):
                    mm(ps_q,
                       qtwq_s[:, cq:cq + 2, 0:BK],
                       qtwq_s[:, cq:cq + 2, BK + hh * 512:BK + (hh + 1) * 512],
                       start=(cq == 0), stop=(cq == QC - 2), perf_mode=DR)
                nc.vector.scalar_tensor_tensor(
                    out=hq_s[:, hs], in0=ps_q, scalar=0.0,
                    in1=b1b_s[:, hs], op0=BYPASS, op1=ADD)

            for t in range(NT):
                emit_closer(t, ps0[t])
            vt_cur = vt_next

            # ---- chunks 1..8
            for ch in range(1, NCH):
                if ch + 1 < NCH:
                    vt_next = vt_chunk_tiles(ch + 1)
                for t in range(NT):
                    rt = ch * NT + t
                    ps = pspool.tile([128, HID], F32, tag="ps", bufs=3)
                    emit_vmms(t, ps)
                    emit_closer(rt, ps)
                vt_cur = vt_next

            # ---- tail: only tile 26 (bk 92:96) remains
            flush_logits(ls_b, "ls_tb2", NRT_A, NRT)
            softmax_head(64, BK, False, True, selap=selm_s)
            rcp = small.tile([BL, N], F32)
            nc.vector.reciprocal(rcp, sums_ps)
            rexp_ps = pspool.tile([BK, N], F32, tag="sm", bufs=2)
            mm(rexp_ps, selbt_s, rcp, start=True, stop=True)
            w96 = small.tile([BK, N], F32)
            nc.vector.scalar_tensor_tensor(
                out=w96, in0=e96, scalar=0.0, in1=rexp_ps,
                op0=BYPASS, op1=MULT)
            nc.sync.dma_start(
                out=out.rearrange("b k n o -> (b k) (n o)"), in_=w96)

    nc.compile()
    return nc


def _get_nc():
    global _NC_CACHE
    if _NC_CACHE is None:
        _NC_CACHE = _build_nc()
    return _NC_CACHE


def _prepare_in_maps(inputs):
    v = np.asarray(inputs["v"], dtype=np.float32)
    q = np.asarray(inputs["q"], dtype=np.float32)
    W1 = np.asarray(inputs["W1"], dtype=np.float32)
    g1 = np.float64(np.asarray(inputs["g1"]))
    b1 = np.asarray(inputs["b1"], dtype=np.float32)
    W2 = np.asarray(inputs["W2"], dtype=np.float32)
    g2 = np.float64(np.asarray(inputs["g2"]))
    # b2 is a scalar added to every logit -> cancels in softmax over k.

    W1e = ((g1 / np.linalg.norm(W1.astype(np.float64))) * W1).astype(np.float32)
    W2e = ((g2 / np.linalg.norm(W2.astype(np.float64))) * W2).astype(np.float32)

    BF = ml_dtypes.bfloat16
    F8 = ml_dtypes.float8_e4m3   # TRN FP8_EXP4 (max ±240, has inf)
    # partition-major repacks: [..., 128 p, chunk, inner]
    w1vt = np.ascontiguousarray(                       # [128, 16, 1024]
        (W1e[:, :VD] * S1).T.reshape(CC, 128, HID).transpose(1, 0, 2)).astype(F8)
    w1qt = (W1e[:, VD:] * S1).T.reshape(QC, 128, HID).transpose(1, 0, 2)  # [128, 8, 1024]
    r = np.arange(R)
    oneh = (np.arange(BK)[:, None] == (r // N)[None, :]).astype(np.float32)
    selb = (np.arange(BL)[None, :] == (np.arange(BK) // K)[:, None]).astype(np.float32)

    w2bf = np.broadcast_to((W2e.reshape(1, HID) * (1.0 / S1)), (128, HID))
    b1bf = np.broadcast_to((b1.reshape(1, HID) * S1), (BK, HID))
    sel = np.zeros((BK, BL + BK + BL), dtype=np.float32)
    sel[:, 0:BL] = selb
    sel[0:BL, BL:BL + BK] = selb.T
    sel[92:BK, BL + BK:BL + BK + BL] = selb[92:BK]

    shared = dict(w1vt=w1vt, oneh=oneh,
                  w2b=np.ascontiguousarray(w2bf).astype(BF),
                  b1b=np.ascontiguousarray(b1bf).astype(BF), sel=sel)
    in_maps = []
    for c in range(NCORES):
        vl = v[c * BL:(c + 1) * BL].reshape(R, VD)
        # vt[ch, p, cc, r_in_chunk] = v[ch*RC + r, cc*128 + p]
        vt4 = np.ascontiguousarray(
            vl.T.reshape(CC, 128, NCH, RC).transpose(2, 1, 0, 3)).astype(F8)
        ql = q[c * BL:(c + 1) * BL].reshape(BK, QD)
        qt3 = ql.T.reshape(QC, 128, BK).transpose(1, 0, 2)   # [128, 8, 96]
        qtwq = np.concatenate([qt3, w1qt], axis=2)           # [128, 8, 1120]
        in_maps.append(dict(vth=np.ascontiguousarray(vt4[:2]),
                            vtr=np.ascontiguousarray(vt4[2:]),
                            qtwq=np.ascontiguousarray(qtwq).astype(F8),
                            **shared))
    return in_maps


def kernel(**inputs) -> np.ndarray:
    in_maps = _prepare_in_maps(inputs)
    nc = _get_nc()
    res = run_bass_kernel_spmd(nc, in_maps, list(range(NCORES)))
    outs = [res.results[c]["out"].reshape(BL, K, N, 1) for c in range(NCORES)]
    return np.concatenate(outs, axis=0)


# revision 10
# speedup vs baseline: 1.0138x; 1.0138x over previous
"""Trainium2 Bass kernel for nn_BigAttention (weight-norm MLP + softmax-over-k).

Math (per the reference):
    W1e = g1 * W1 / ||W1||_F          [1024, 3072]
    W2e = g2 * W2 / ||W2||_F          [1, 1024]
    hv  = v @ W1e[:, :2048].T         [B,K,N,1024]
    hq  = q @ W1e[:, 2048:].T         [B,K,1024]
    joint  = relu(hv + hq + b1)
    logits = joint @ W2e.T  (+ b2, which cancels in the softmax over k)
    out = softmax(logits, axis=K)     [B,K,N,1]

Sharding: data-parallel over batch, 8 batches per core; weights replicated.

The dominant v-matmuls and the hq matmuls run fp8 e4m3 with
perf_mode=DoubleRow: two 128-deep contraction chunks fuse into one PE
instruction streaming 2 cols/cycle (~2x bf16). W1 is scaled by S1 host-side
so its tiny entries survive e4m3; hq/b1 carry the same scale and W2 carries
1/S1 (relu commutes with positive scaling). The one-hot hq-add closers
contract only 96 bk-partitions, where DoubleRow cannot cut streamed columns
-- they stay bf16 (FWL-eligible stationary).

Per-core device program (rows r = (b_local, k, n) flattened, R = 3456):
  - warmup: a short burst of dummy matmuls on zeroed SBUF rides the initial
    DMA wait so the PE's HAM clock-gate is at full rate when real data lands.
  - hq[96, 1024] via 8 DoubleRow matmuls; b1 added on the PSUM->SBUF copy.
  - main: per 128-row tile, PSUM[row, hidden 1024] accumulates 8 paired
    v^T-chunk DoubleRow matmuls plus two bf16 one-hot closers adding
    hq[bk(row), :].
  - epilogue per tile: one DVE scalar_tensor_tensor computes
    (PSUM max 0) * w2_broadcast with accum_out = per-row sum = the logit.
  - softmax over k, three stages so only bk 92..95 remain after the last
    tile: logits go [128, t] -> StreamTranspose -> linear DRAM ->
    [96 (b,k), 36 n] SBUF; exp on ACT; per-(b,n) sums accumulate on the PE
    across stages; reciprocal broadcast back via one-hot matmul; final
    scale on DVE; one strided DMA writes the [8,12,36,1] output slice.

All heavy inputs are host-repacked "partition-major" so every big DMA is 128
contiguous runs (one per partition). DMA issue order matches consumption
order, with the first v/W1v pieces split small so the first real matmul can
start as early as possible.
"""

import ml_dtypes
import numpy as np

import concourse.bacc as bacc
import concourse.mybir as mybir
import concourse.tile as tile
from concourse.bass_utils import run_bass_kernel_spmd

F32 = mybir.dt.float32
NCORES = 8
B, K, N = 64, 12, 36
VD, QD, HID = 2048, 1024, 1024
BL = B // NCORES              # local batches per core
R = BL * K * N                # 3456 rows per core
BK = BL * K                   # 96 (b,k) groups per core
CC = VD // 128                # 16 contraction chunks over v-dim
QC = QD // 128                # 8 contraction chunks over q-dim
RC = 384                      # rows per DMA chunk (9 chunks)
NCH = R // RC
RT = 128                      # rows per PSUM tile
NT = RC // RT
NRT = R // RT                 # 27 row tiles

_NC_CACHE = None

MM_DT = mybir.dt.float32r
BF16 = mybir.dt.bfloat16
FP8 = mybir.dt.float8e4
S1 = 2048.0

# cc-chunk DMA granularity per v chunk: chunk 0 lands in four small pieces
# so the first matmul pair can start early; later chunks use two fat DMAs.
VSPLITS0 = (2, 2, 2, 2, 4, 4)
VSPLITS = (8, 8)
WSPLITS = (2, 2, 4, 4, 4)     # w1v chunk groups (first ones small)


def _splits_index(splits):
    idx = {}
    base = 0
    for j, g in enumerate(splits):
        for o in range(g):
            idx[base + o] = (j, o)
        base += g
    return idx


def _build_nc():
    nc = bacc.Bacc("TRN2", target_bir_lowering=False, debug=False,
                   num_devices=NCORES)

    DR = mybir.MatmulPerfMode.DoubleRow

    def mm(out, lhsT, rhs, **kw):
        nc.tensor.matmul(out, lhsT, rhs, **kw)

    w1vt = nc.dram_tensor("w1vt", [128, CC, HID], FP8, kind="ExternalInput").ap()
    # qt and W1q^T packed along the free dim: [:, cq, 0:96]=q^T, [:, cq, 96:1120]=W1q^T
    qtwq = nc.dram_tensor("qtwq", [128, QC, BK + HID], FP8, kind="ExternalInput").ap()
    # small constants: W2e/S1 replicated (bf16), b1*S1 replicated (bf16),
    # softmax selection matrices (fp32)
    w2b = nc.dram_tensor("w2b", [128, HID], BF16, kind="ExternalInput").ap()
    b1b = nc.dram_tensor("b1b", [BK, HID], BF16, kind="ExternalInput").ap()
    sel = nc.dram_tensor("sel", [BK, BL + BK + BL], F32, kind="ExternalInput").ap()
    # v is split: the first two chunks ride with the weights at the front of
    # the upload order; the bulk uploads last, hidden under early compute.
    vth = nc.dram_tensor("vth", [2, 128, CC, RC], FP8, kind="ExternalInput").ap()
    vtr = nc.dram_tensor("vtr", [NCH - 2, 128, CC, RC], FP8, kind="ExternalInput").ap()
    out = nc.dram_tensor("out", [BL, K, N, 1], F32, kind="ExternalOutput").ap()

    MAX = mybir.AluOpType.max
    MULT = mybir.AluOpType.mult
    BYPASS = mybir.AluOpType.bypass
    ADD = mybir.AluOpType.add

    vidx0 = _splits_index(VSPLITS0)
    vidx = _splits_index(VSPLITS)
    widx = _splits_index(WSPLITS)

    with tile.TileContext(nc) as tc:
        with tc.tile_pool(name="const", bufs=1) as cpool, \
             tc.tile_pool(name="wv", bufs=1) as wvpool, \
             tc.tile_pool(name="vtp", bufs=2) as vtpool, \
             tc.tile_pool(name="work", bufs=3) as work, \
             tc.tile_pool(name="small", bufs=1) as small, \
             tc.tile_pool(name="dram", bufs=1, space="DRAM") as dpool, \
             tc.tile_pool(name="psum", bufs=4, space="PSUM") as pspool:

            # ---- PE warmup burst: ~10 dummy matmuls on zeroed SBUF keep the
            # HAM clock-gate ramping while the first real operands upload.
            wdum = small.tile([128, 2, 128], FP8)
            nc.vector.memset(wdum, 0.0)
            xdum = small.tile([128, 2, 512], FP8)
            nc.vector.memset(xdum, 0.0)
            psd = pspool.tile([128, 512], F32, tag="sm", bufs=2)
            for _ in range(14):
                mm(psd, wdum, xdum, start=True, stop=True, perf_mode=DR)

            # ---- startup DMAs; issue order matches consumption order: the
            # first matmul pair needs v chunk 0 piece 0 + w1v group 0.
            def vt_chunk_tiles(ch):
                src_ap = vth[ch] if ch < 2 else vtr[ch - 2]
                splits = VSPLITS0 if ch == 0 else VSPLITS
                tiles = []
                base = 0
                for j, g in enumerate(splits):
                    t = vtpool.tile([128, g, RC], FP8, tag=f"vt{j}_{g}")
                    nc.sync.dma_start(out=t, in_=src_ap[:, base:base + g, :])
                    tiles.append(t)
                    base += g
                return tiles

            wv_g = []

            def wv_group(j):
                base = sum(WSPLITS[:j])
                g = WSPLITS[j]
                t = wvpool.tile([128, g, HID], FP8, tag=f"wvg{j}")
                nc.scalar.dma_start(out=t, in_=w1vt[:, base:base + g, :])
                wv_g.append(t)

            vt_cur = vt_chunk_tiles(0)
            for j in range(len(WSPLITS)):
                wv_group(j)

            qtwq_s = cpool.tile([128, QC, BK + HID], FP8)
            nc.sync.dma_start(out=qtwq_s, in_=qtwq)

            vt_next = vt_chunk_tiles(1)

            w2b_s = cpool.tile([128, HID], BF16)
            nc.scalar.dma_start(out=w2b_s, in_=w2b)
            b1b_s = cpool.tile([BK, HID], BF16)
            nc.gpsimd.dma_start(out=b1b_s, in_=b1b)
            sel_s = cpool.tile([BK, BL + BK + BL], F32)
            nc.gpsimd.dma_start(out=sel_s, in_=sel)

            # fp32r one-hot for the hq-add closers (bf16 here was measured
            # to slow the whole PE stream ~20% via weight-path mode thrash).
            # Pure structure -- built on the idle gpsimd engine instead of
            # spending 1.3MB of the HBM-bound startup feed:
            # oneh[bk, r] = 1 iff 36*bk <= r <= 36*bk + 35
            oneh_s = cpool.tile([BK, R], MM_DT)
            onehw = cpool.tile([BK, R], F32, name="onehw")
            oneh3 = onehw.rearrange("p (g j) -> p g j", g=BK, j=N)
            nc.gpsimd.memset(onehw, 1.0)
            # two is_ge selects (walrus codegen rejects is_le):
            # keep where p - g >= 0 AND g - p >= 0, i.e. g == p
            nc.gpsimd.affine_select(
                out=oneh3, in_=oneh3, compare_op=mybir.AluOpType.is_ge,
                fill=0.0, base=0, channel_multiplier=1,
                pattern=[[-1, BK], [0, N]])
            nc.gpsimd.affine_select(
                out=oneh3, in_=oneh3, compare_op=mybir.AluOpType.is_ge,
                fill=0.0, base=0, channel_multiplier=-1,
                pattern=[[1, BK], [0, N]])
            # final copy re-types the result fp32r for the closer matmuls
            # (the BIR verifier requires an fp32r-rounding producer)
            nc.gpsimd.tensor_copy(oneh_s, onehw)

            selb_s = sel_s[:, 0:BL]
            selbt_s = sel_s[0:BL, BL:BL + BK]
            # selb with bk rows < 92 zeroed: lets the tail sums matmul
            # contract from base partition 64 (hw allows 0/32/64 only)
            selm_s = sel_s[:, BL + BK:BL + BK + BL]

            # per-row logits, laid out [p, rt] with row = rt*128 + p, split
            # into two tiles so earlier flushes hide under the main loop.
            # 32 columns (StreamTranspose needs 32x32 blocks).
            NRT_A = 18   # 18*128 rows = 64 (b,k) groups — a 32-aligned bk split
            ls_a = cpool.tile([128, 32], F32)
            nc.vector.memset(ls_a, 0.0)
            ls_b = cpool.tile([128, 32], F32)
            nc.vector.memset(ls_b, 0.0)
            lg = dpool.tile([R], F32)
            lg2 = lg.rearrange("(t p) -> t p", t=NRT, p=128)

            def flush_logits(ls, ls_t_name, t0, t1):
                # ls[p, t - t0] holds L[t*128 + p] for t in [t0, t1)
                ls_t = cpool.tile([128, 32], F32, name=ls_t_name)
                nc.vector.transpose(ls_t, ls)
                # four rings generate descriptors in parallel (the ~0.7us
                # DIRECT2D gen per DMA is the dominant serial tail cost)
                engs = (nc.sync, nc.scalar, nc.gpsimd, nc.scalar)
                for i in range(4):
                    engs[i].dma_start(
                        out=lg2[t0:t1, 32 * i:32 * i + 32],
                        in_=ls_t[32 * i:32 * i + (t1 - t0), :])

            hq_s = cpool.tile([BK, HID], MM_DT)
            s96 = small.tile([BK, N], F32)
            e96 = small.tile([BK, N], F32)
            sums_ps = pspool.tile([BL, N], F32, tag="sm", bufs=2)

            def softmax_head(bk0, bk1, start, stop, selap=None, dma0=None):
                # exp + partial per-(b,n) sums for bk rows [bk0, bk1)
                d0 = bk0 if dma0 is None else dma0
                nc.sync.dma_start(
                    out=s96[d0:bk1, :],
                    in_=lg.rearrange("(bk n) -> bk n", n=N)[d0:bk1, :])
                nc.scalar.activation(e96[bk0:bk1, :], s96[bk0:bk1, :],
                                     mybir.ActivationFunctionType.Exp)
                sl = selb_s if selap is None else selap
                mm(sums_ps, sl[bk0:bk1, :], e96[bk0:bk1, :],
                   start=start, stop=stop)

            def emit_vmms(t, ps):
                # fp8 DoubleRow: each matmul contracts TWO 128-deep v chunks
                # (lhsT [128, 2, 128 rows], rhs [128, 2, 512]) at 2 cols/cycle.
                for cc in range(0, CC, 2):
                    vj, vo = (vidx0 if vt_cur is vt_c0 else vidx)[cc]
                    lhsT = vt_cur[vj][:, vo:vo + 2, t * RT:(t + 1) * RT]
                    wj, wo = widx[cc]
                    wvc = wv_g[wj][:, wo:wo + 2, :]
                    mm(ps[:, 0:512], lhsT, wvc[:, :, 0:512],
                       start=(cc == 0), stop=False, perf_mode=DR)
                    mm(ps[:, 512:1024], lhsT, wvc[:, :, 512:1024],
                       start=(cc == 0), stop=False, perf_mode=DR)

            def emit_closer(rt, ps):
                oh = oneh_s[:, rt * RT:(rt + 1) * RT]
                mm(ps[:, 0:512], oh, hq_s[:, 0:512], start=False, stop=True)
                mm(ps[:, 512:1024], oh, hq_s[:, 512:1024],
                   start=False, stop=True)
                relu_w2 = work.tile([128, HID], F32, tag="relu_w2")
                ls, col = (ls_a, rt) if rt < NRT_A else (ls_b, rt - NRT_A)
                nc.vector.scalar_tensor_tensor(
                    out=relu_w2, in0=ps, scalar=0.0, in1=w2b_s,
                    op0=MAX, op1=MULT,
                    accum_out=ls[:, col:col + 1])
                if rt == NRT_A - 1:
                    # flush + softmax head for bk 0:64 under the main loop
                    flush_logits(ls_a, "ls_ta", 0, NRT_A)
                    softmax_head(0, 64, True, False)
                elif rt == NRT - 2:
                    # bk 64:92 are complete through tile 25; only bk 92:96
                    # remain for the post-loop tail
                    flush_logits(ls_b, "ls_tb", NRT_A, NRT - 1)
                    softmax_head(64, 92, False, False)

            # ---- chunk 0: v-matmuls for tiles 0..2 first, then hq (its DMAs
            # arrive under the v work), then the deferred closers.
            vt_c0 = vt_cur
            ps0 = []
            for t in range(NT):
                ps = pspool.tile([128, HID], F32, tag="ps", bufs=3)
                emit_vmms(t, ps)
                ps0.append(ps)

            # hq[96, 1024] via fp8 DoubleRow over q-chunk pairs (out partition
            # 96 = lhsT free 192 / 2); b1 (scaled) added on the PSUM -> SBUF
            # copy, stored bf16 for the closers.
            for hh in range(2):
                hs = slice(hh * 512, (hh + 1) * 512)
                ps_q = pspool.tile([BK, 512], F32, tag="sm", bufs=2,
                                   name=f"hq_ps{hh}")
                for cq in range(0, QC, 2):
                    mm(ps_q,
                       qtwq_s[:, cq:cq + 2, 0:BK],
                       qtwq_s[:, cq:cq + 2, BK + hh * 512:BK + (hh + 1) * 512],
                       start=(cq == 0), stop=(cq == QC - 2), perf_mode=DR)
                nc.vector.scalar_tensor_tensor(
                    out=hq_s[:, hs], in0=ps_q, scalar=0.0,
                    in1=b1b_s[:, hs], op0=BYPASS, op1=ADD)

            for t in range(NT):
                emit_closer(t, ps0[t])
            vt_cur = vt_next

            # ---- chunks 1..8
            for ch in range(1, NCH):
                if ch + 1 < NCH:
                    vt_next = vt_chunk_tiles(ch + 1)
                for t in range(NT):
                    rt = ch * NT + t
                    ps = pspool.tile([128, HID], F32, tag="ps", bufs=3)
                    emit_vmms(t, ps)
                    emit_closer(rt, ps)
                vt_cur = vt_next

            # ---- tail: only tile 26 (bk 92:96) remains
            flush_logits(ls_b, "ls_tb2", NRT_A, NRT)
            softmax_head(64, BK, False, True, selap=selm_s, dma0=92)
            rcp = small.tile([BL, N], F32)
            nc.vector.reciprocal(rcp, sums_ps)
            rexp_ps = pspool.tile([BK, N], F32, tag="sm", bufs=2)
            mm(rexp_ps, selbt_s, rcp, start=True, stop=True)
            w96 = small.tile([BK, N], F32)
            nc.vector.scalar_tensor_tensor(
                out=w96, in0=e96, scalar=0.0, in1=rexp_ps,
                op0=BYPASS, op1=MULT)
            nc.sync.dma_start(
                out=out.rearrange("b k n o -> (b k) (n o)"), in_=w96)

    nc.compile()
    return nc


def _get_nc():
    global _NC_CACHE
    if _NC_CACHE is None:
        _NC_CACHE = _build_nc()
    return _NC_CACHE


def _prepare_in_maps(inputs):
    v = np.asarray(inputs["v"], dtype=np.float32)
    q = np.asarray(inputs["q"], dtype=np.float32)
    W1 = np.asarray(inputs["W1"], dtype=np.float32)
    g1 = np.float64(np.asarray(inputs["g1"]))
    b1 = np.asarray(inputs["b1"], dtype=np.float32)
    W2 = np.asarray(inputs["W2"], dtype=np.float32)
    g2 = np.float64(np.asarray(inputs["g2"]))
    # b2 is a scalar added to every logit -> cancels in softmax over k.

    W1e = ((g1 / np.linalg.norm(W1.astype(np.float64))) * W1).astype(np.float32)
    W2e = ((g2 / np.linalg.norm(W2.astype(np.float64))) * W2).astype(np.float32)

    BF = ml_dtypes.bfloat16
    F8 = ml_dtypes.float8_e4m3   # TRN FP8_EXP4 (max ±240, has inf)
    # partition-major repacks: [..., 128 p, chunk, inner]
    w1vt = np.ascontiguousarray(                       # [128, 16, 1024]
        (W1e[:, :VD] * S1).T.reshape(CC, 128, HID).transpose(1, 0, 2)).astype(F8)
    w1qt = (W1e[:, VD:] * S1).T.reshape(QC, 128, HID).transpose(1, 0, 2)  # [128, 8, 1024]
    selb = (np.arange(BL)[None, :] == (np.arange(BK) // K)[:, None]).astype(np.float32)

    w2bf = np.broadcast_to((W2e.reshape(1, HID) * (1.0 / S1)), (128, HID))
    b1bf = np.broadcast_to((b1.reshape(1, HID) * S1), (BK, HID))
    sel = np.zeros((BK, BL + BK + BL), dtype=np.float32)
    sel[:, 0:BL] = selb
    sel[0:BL, BL:BL + BK] = selb.T
    sel[92:BK, BL + BK:BL + BK + BL] = selb[92:BK]

    shared = dict(w1vt=w1vt,
                  w2b=np.ascontiguousarray(w2bf).astype(BF),
                  b1b=np.ascontiguousarray(b1bf).astype(BF), sel=sel)
    in_maps = []
    for c in range(NCORES):
        vl = v[c * BL:(c + 1) * BL].reshape(R, VD)
        # vt[ch, p, cc, r_in_chunk] = v[ch*RC + r, cc*128 + p]
        vt4 = np.ascontiguousarray(
            vl.T.reshape(CC, 128, NCH, RC).transpose(2, 1, 0, 3)).astype(F8)
        ql = q[c * BL:(c + 1) * BL].reshape(BK, QD)
        qt3 = ql.T.reshape(QC, 128, BK).transpose(1, 0, 2)   # [128, 8, 96]
        qtwq = np.concatenate([qt3, w1qt], axis=2)           # [128, 8, 1120]
        in_maps.append(dict(vth=np.ascontiguousarray(vt4[:2]),
                            vtr=np.ascontiguousarray(vt4[2:]),
                            qtwq=np.ascontiguousarray(qtwq).astype(F8),
                            **shared))
    return in_maps


def kernel(**inputs) -> np.ndarray:
    in_maps = _prepare_in_maps(inputs)
    nc = _get_nc()
    res = run_bass_kernel_spmd(nc, in_maps, list(range(NCORES)))
    outs = [res.results[c]["out"].reshape(BL, K, N, 1) for c in range(NCORES)]
    return np.concatenate(outs, axis=0)
